# revision 2
# baseline (speedup 1.0000x reference)
"""TextLSTM kernel for 8 Trainium2 NeuronCores.

Data-parallel over batch: each of the 8 cores runs the full model on a
512-row batch shard.

Per-core pipeline (feature-major LSTM):
  1. Embedding gather: indirect-DMA 2560 rows of Emb (t-major token order)
     into SBUF batch-major, PE-transpose 128x128 blocks into feature-major
     xT[t] tiles (cast bf16).
  2. LSTM recurrence, 5 steps: gates[4H, 512b] = W.T @ [h; x_t] with mixed
     precision: the x-part (K=512) runs as bf16 128x128 matmuls with weights
     pre-scaled by 2^13; the recurrent h-part (K=1024) runs as fp8-e4m3
     DoubleRow matmuls (two 128-deep k-planes per instruction, 2x PE rate)
     with weights scaled by 2^6 and h re-quantized to fp8 (x2^7) each step,
     so both accumulate into one PSUM at a common 2^13 scale. The scalar
     activation dequantizes via its scale port (sigmoid/tanh(psum/8192+b)).
     Cell math fp32 on VectorE; c fp32; h8 double-buffered.
  3. Projection: out[512b, 32000v] = h.T @ WoutT streamed per 512-col vocab
     tile (bf16 weights, fp32 PSUM/output) from the last step's bf16 h.

The fp8 h-part is safe for the 2e-2 rel-err gate: the recurrent term carries
~16% of the preactivation signal, so e4m3's ~2.6%/operand quantization noise
lands at ~0.9% end-to-end (measured 0.0087 vs 0.0034 all-bf16).

Weights are pre-transposed/tiled/cast/scaled on the host; biases are all
zero per the problem spec (gate biases are still applied on-device via the
activation bias port; bout is added on host only if nonzero).
"""

import os
import sys

import numpy as np
import ml_dtypes

for _p in ("/opt/trn_rl_repo", "/root/.axon_site/_ro/trn_rl_repo"):
    if os.path.isdir(_p) and _p not in sys.path:
        sys.path.append(_p)

from concourse import bacc, mybir
import concourse.tile as tile
from concourse.bass import IndirectOffsetOnAxis
from concourse.bass_utils import run_bass_kernel_spmd
from concourse.masks import make_identity

P = 128
B, T, E, H, V = 4096, 5, 512, 1024, 32000
NCORES = 8
BS = B // NCORES          # 512 batch rows per core
NTOK = BS * T             # 2560 gathered tokens per core
NG = NTOK // P            # 20 gather tiles of 128 tokens
KH = H // P               # 8 k-tiles over h
KE = E // P               # 4 k-tiles over x
NJ = H // P               # 8 hidden-dim tiles
VN = 512                  # vocab tile width
VT = (V + VN - 1) // VN   # 63 vocab tiles (last one 256 wide)
VPAD = VT * VN            # 32256
NBT = BS // P             # 4 batch tiles

SW = 64.0                 # fp8 scale on h-part weights
SH = 128.0                # fp8 scale on h
SXW = SW * SH             # 8192: common PSUM scale; x-weights pre-scaled by it

F32 = mybir.dt.float32
BF16 = mybir.dt.bfloat16
FP8 = mybir.dt.float8e4
I32 = mybir.dt.int32
AF = mybir.ActivationFunctionType
DR = mybir.MatmulPerfMode.DoubleRow

_BF = ml_dtypes.bfloat16
_F8 = ml_dtypes.float8_e4m3

_CACHE = {}
LAST_RESULTS = None


def _build():
    nc = bacc.Bacc("TRN2", target_bir_lowering=False, debug=False,
                   num_devices=NCORES)

    idx_d = nc.dram_tensor("idx", [P, NG], I32, kind="ExternalInput")
    emb_d = nc.dram_tensor("emb", [V, E], BF16, kind="ExternalInput")
    wx_d = nc.dram_tensor("wx", [P, KE, 4 * H], BF16, kind="ExternalInput")
    wh8_d = nc.dram_tensor("wh8", [P, KH, 4 * H], FP8, kind="ExternalInput")
    bias_d = nc.dram_tensor("bias", [P, 4 * H // P], F32, kind="ExternalInput")
    wo_d = nc.dram_tensor("wo", [VT, P, KH * VN], BF16, kind="ExternalInput")
    out_d = nc.dram_tensor("out", [BS, V], F32, kind="ExternalOutput")

    with tile.TileContext(nc) as tc:
        with (
            tc.tile_pool(name="const", bufs=1) as cpool,
            tc.tile_pool(name="gather", bufs=6) as gpool,
            tc.tile_pool(name="work", bufs=2) as wpool,
            tc.tile_pool(name="woutp", bufs=3) as wopool,
            tc.tile_pool(name="outp", bufs=4) as opool,
            tc.tile_pool(name="psum", bufs=8, space="PSUM") as pspool,
        ):
            ident = cpool.tile([P, P], BF16, tag="ident")
            make_identity(nc, ident[:])

            # persistent SBUF state
            wx_sb = cpool.tile([P, KE, 4 * H], BF16, tag="wx")
            wh8_sb = cpool.tile([P, KH, 4 * H], FP8, tag="wh8")
            bias_sb = cpool.tile([P, 4 * H // P], F32, tag="bias")
            h8_sb = cpool.tile([P, 2, KH, BS], FP8, tag="h8")
            hbf_sb = cpool.tile([P, KH, BS], BF16, tag="hbf")
            c_sb = cpool.tile([P, NJ, BS], F32, tag="c")
            xt_sb = cpool.tile([P, T, KE, BS], BF16, tag="xt")
            idx_sb = cpool.tile([P, NG], I32, tag="idx")

            nc.sync.dma_start(out=idx_sb[:], in_=idx_d.ap())
            nc.sync.dma_start(out=bias_sb[:], in_=bias_d.ap())
            # x-part weights first: they gate the t=0 matmuls, the h-part
            # loads overlap with t=0 compute.
            for kt in range(KE):
                nc.sync.dma_start(out=wx_sb[:, kt, :], in_=wx_d.ap()[:, kt, :])
            for kt in range(KH):
                nc.sync.dma_start(out=wh8_sb[:, kt, :], in_=wh8_d.ap()[:, kt, :])

            # all embedding gathers issued upfront; they pipeline on the
            # dynamic DMA queue well ahead of the recurrence consuming them.
            xgs = []
            for g in range(NG):
                xg = gpool.tile([P, E], BF16, tag="xg")
                nc.gpsimd.indirect_dma_start(
                    out=xg[:],
                    out_offset=None,
                    in_=emb_d.ap(),
                    in_offset=IndirectOffsetOnAxis(ap=idx_sb[:, g:g + 1], axis=0),
                )
                xgs.append(xg)

            # PE-transpose one step's gather tiles into feature-major
            def emit_transposes(tt):
                for bb in range(NBT):
                    xg = xgs[tt * NBT + bb]
                    for e in range(KE):
                        ps_tr = pspool.tile([P, P], BF16, tag="ps",
                                            name="ps_tr")
                        nc.tensor.transpose(
                            ps_tr[:], xg[:, e * P:(e + 1) * P], ident[:])
                        nc.vector.tensor_copy(
                            out=xt_sb[:, tt, e, bb * P:(bb + 1) * P],
                            in_=ps_tr[:])

            # ---- LSTM recurrence ----
            emit_transposes(0)
            for t in range(T):
                rbuf, wbuf = t % 2, (t + 1) % 2
                for j in range(NJ):
                    # next step's transposes go mid-stream, where PSUM slots
                    # are freshly recycled — not at the step boundary where
                    # they'd contend with the previous step's gate drains
                    if j == 1 and t + 1 < T:
                        emit_transposes(t + 1)
                    gate_ps = []
                    for gi in range(4):
                        ps = pspool.tile([P, VN], F32, tag="ps")
                        col = gi * H + j * P
                        # x-part: bf16, rhs ready immediately so PE enters
                        # the step while the previous step's h is finishing
                        for k in range(KE):
                            nc.tensor.matmul(
                                ps[:],
                                lhsT=wx_sb[:, k, col:col + P],
                                rhs=xt_sb[:, t, k, :],
                                start=(k == 0),
                                stop=(t == 0 and k == KE - 1),
                            )
                        # h-part: fp8 DoubleRow, two k-planes per matmul
                        if t > 0:
                            for m in range(KH // 2):
                                nc.tensor.matmul(
                                    ps[:],
                                    lhsT=wh8_sb[:, 2 * m:2 * m + 2,
                                                col:col + P],
                                    rhs=h8_sb[:, rbuf, 2 * m:2 * m + 2, :],
                                    start=False,
                                    stop=(m == KH // 2 - 1),
                                    perf_mode=DR,
                                )
                        gate_ps.append(ps)

                    bcol = lambda gi: bias_sb[:, gi * NJ + j:gi * NJ + j + 1]
                    f_sb = wpool.tile([P, BS], F32, tag="f")
                    i_sb = wpool.tile([P, BS], F32, tag="i")
                    g_sb = wpool.tile([P, BS], F32, tag="g")
                    o_sb = wpool.tile([P, BS], F32, tag="o")
                    dq = 1.0 / SXW
                    nc.scalar.activation(f_sb[:], gate_ps[0][:], AF.Sigmoid,
                                         bias=bcol(0), scale=dq)
                    nc.scalar.activation(i_sb[:], gate_ps[1][:], AF.Sigmoid,
                                         bias=bcol(1), scale=dq)
                    nc.scalar.activation(g_sb[:], gate_ps[2][:], AF.Tanh,
                                         bias=bcol(2), scale=dq)
                    nc.scalar.activation(o_sb[:], gate_ps[3][:], AF.Sigmoid,
                                         bias=bcol(3), scale=dq)

                    if t == 0:
                        nc.vector.tensor_mul(out=c_sb[:, j, :], in0=i_sb[:],
                                             in1=g_sb[:])
                    else:
                        # in-place: c *= f; g_sb = i*g; c += g_sb
                        nc.vector.tensor_mul(out=c_sb[:, j, :], in0=f_sb[:],
                                             in1=c_sb[:, j, :])
                        nc.vector.tensor_mul(out=g_sb[:], in0=i_sb[:],
                                             in1=g_sb[:])
                        nc.vector.tensor_add(out=c_sb[:, j, :],
                                             in0=c_sb[:, j, :], in1=g_sb[:])
                    th = wpool.tile([P, BS], F32, tag="th")
                    nc.scalar.activation(th[:], c_sb[:, j, :], AF.Tanh)
                    if t < T - 1:
                        # h = o*tanh(c) in fp32 (in-place into th), then
                        # requantize to fp8 (x128) for the next step's
                        # DoubleRow rhs
                        nc.vector.tensor_mul(out=th[:], in0=o_sb[:],
                                             in1=th[:])
                        nc.scalar.activation(h8_sb[:, wbuf, j, :], th[:],
                                             AF.Copy, scale=SH)
                    else:
                        # last step: h feeds only the projection, keep bf16
                        nc.vector.tensor_mul(out=hbf_sb[:, j, :],
                                             in0=o_sb[:], in1=th[:])

            # ---- output projection ----
            QW = KH * VN // 4  # wout tile loaded in 4 quarters for overlap
            for vt in range(VT):
                vn = min(VN, V - vt * VN)
                wo_sb = wopool.tile([P, KH * VN], BF16, tag="wo")
                for q in range(4):
                    nc.sync.dma_start(out=wo_sb[:, q * QW:(q + 1) * QW],
                                      in_=wo_d.ap()[vt][:, q * QW:(q + 1) * QW])
                for bt in range(NBT):
                    ps = pspool.tile([P, VN], F32, tag="ps")
                    for k in range(KH):
                        nc.tensor.matmul(
                            ps[:, :vn],
                            lhsT=hbf_sb[:, k, bt * P:(bt + 1) * P],
                            rhs=wo_sb[:, k * VN:k * VN + vn],
                            start=(k == 0),
                            stop=(k == KH - 1),
                        )
                    ot = opool.tile([P, VN], F32, tag="ot")
                    nc.vector.tensor_copy(out=ot[:, :vn], in_=ps[:, :vn])
                    # logit writes go out on the ACT HWDGE queue so they
                    # don't contend with the wout reads on the sync queue
                    nc.scalar.dma_start(
                        out=out_d.ap()[bt * P:(bt + 1) * P,
                                       vt * VN:vt * VN + vn],
                        in_=ot[:, :vn])

    nc.compile()
    return nc


def get_nc():
    if "nc" not in _CACHE:
        _CACHE["nc"] = _build()
    return _CACHE["nc"]


def _prep_shared(Emb, WF, WI, WC, WO, bF, bI, bC, bO, Wout):
    emb = np.ascontiguousarray(np.asarray(Emb, dtype=np.float32)).astype(_BF)

    WT = np.concatenate([np.asarray(WF), np.asarray(WI), np.asarray(WC),
                         np.asarray(WO)], 0).astype(np.float32).T  # [1536, 4096]
    wh8 = np.ascontiguousarray(
        (WT[:H] * SW).reshape(KH, P, 4 * H).transpose(1, 0, 2)
    ).astype(_F8)                                               # [128, 8, 4096]
    wx = np.ascontiguousarray(
        (WT[H:] * SXW).reshape(KE, P, 4 * H).transpose(1, 0, 2)
    ).astype(_BF)                                               # [128, 4, 4096]

    b_all = np.concatenate([np.asarray(bF), np.asarray(bI), np.asarray(bC),
                            np.asarray(bO)], 0).astype(np.float32)  # [4096]
    bias = np.ascontiguousarray(b_all.reshape(4 * H // P, P).T)  # [128, 32]

    Wout = np.asarray(Wout, dtype=np.float32)
    wpad = np.zeros((VPAD, H), np.float32)
    wpad[:V] = Wout
    wo = np.ascontiguousarray(
        wpad.reshape(VT, VN, KH, P).transpose(0, 3, 2, 1).reshape(VT, P, KH * VN)
    ).astype(_BF)  # [63, 128, 4096]
    return emb, wx, wh8, bias, wo


def kernel(X, Emb, WF, bF, WI, bI, WC, bC, WO, bO, Wout, bout):
    global LAST_RESULTS
    nc = get_nc()

    emb, wx, wh8, bias, wo = _prep_shared(Emb, WF, WI, WC, WO, bF, bI, bC, bO,
                                          Wout)
    X = np.asarray(X).astype(np.int32)  # [4096, 5]

    in_maps = []
    for c in range(NCORES):
        xs = X[c * BS:(c + 1) * BS]                       # [512, 5]
        idx = np.ascontiguousarray(
            xs.T.reshape(NG, P).T).astype(np.int32)       # [128, 20] t-major
        in_maps.append({"idx": idx, "emb": emb, "wx": wx, "wh8": wh8,
                        "bias": bias, "wo": wo})

    res = run_bass_kernel_spmd(nc, in_maps, core_ids=list(range(NCORES)))
    LAST_RESULTS = res

    out = np.concatenate([res.results[c]["out"] for c in range(NCORES)], 0)
    bout = np.asarray(bout, dtype=np.float32)
    if np.any(bout):
        out = out + bout[None, :]
    return out
